# revision 62
# baseline (speedup 1.0000x reference)
"""Trainium2 Bass kernel for a spectral-normed linear + Ricker-wavelet KAN layer.

Math (per token row x_n in R^1024):
  base  = silu(x_n) @ (base_w/sigma).T + base_b
  th    = tanh(x_n);  u_g = a_g*th + b_g  (a_g = 2.5/s_g, b_g = -t_g/s_g)
  basis = (1 - u^2) * exp(-u^2/2)         (7 wavelets per feature)
  kan   = softshrink(basis_flat @ wavelet_w.T, thr=softplus(soft_threshold))
  out   = (base + kan) * output_scale

Strategy: data-parallel across 8 NeuronCores (2048 tokens each), weights
replicated.  The wavelet matmul (7/8 of the FLOPs) runs in fp8-e4m3 with
MatmulPerfMode.DoubleRow (two 128-deep k-tiles per instruction, 0.5
cycles/row) -- 4x the bf16 matmul rate.  Weights are pre-scaled by 64 so
fp8 stays in its normal range; the epilogue divides back.

Per 512-token super-tile each of the 8 PSUM banks runs ONE accumulation
group:  28 DoubleRow matmuls (64*kan)  ->  DVE reads the bank and writes
t = clamp(kan64, +-64*thr)  ->  PE folds -t back in via a (-I) matmul
-> 8 bf16 base matmuls (64*base) -> stop.  The bank then holds
64*(softshrink(kan) + base) and a single DVE tensor_scalar produces the
f32 output tile.

Elementwise work is split across engines so they all sit near 75% busy:
ACT does tanh / tanh(x/2) (for the 2*silu identity -- keeps every
activation in the single exp_and_others table, zero table reloads) /
big-tile exp / a tunable share of Squares; DVE computes
v' = u^2-1 = (u+1)(u-1) from two 4x-mode tensor_scalar affines and a 2x
tensor_tensor, plus the PSUM epilogue ops; GPSIMD (Pool) runs most of
the v'*e -> fp8 basis products (plain tensor_tensor -- the only ALU op
shape legal on Pool).  SQ_ACT/NB_POOL shift per-chain work between
engines; chain 0 of each super-tile leans on ACT/Pool because DVE is
busy with the previous tile's epilogue then.
"""

import sys

if '/opt/trn_rl_repo' not in sys.path:
    sys.path.insert(0, '/opt/trn_rl_repo')

import numpy as np
import ml_dtypes

import concourse.bass as bass
import concourse.mybir as mybir
import concourse.tile as tile
from concourse import bacc
from concourse.bass_utils import run_bass_kernel_spmd

N_CORES = 8
BATCH, SEQ, IN_F, OUT_F, GRID = 4, 4096, 1024, 1024, 7
NTOK = BATCH * SEQ            # 16384 tokens
TPC = NTOK // N_CORES         # 2048 tokens per core
ST = 512                      # tokens per super-tile
NST = TPC // ST               # 4 super-tiles per core
NIC = IN_F // 128             # 8 input-feature chunks
NPI = NIC // 2                # 4 chunk-pairs (DoubleRow k-tile pairs)
NCH = NST * NPI               # 16 chains (st, icpair) per core
NTT = ST // 128               # 4 token tiles per super-tile
NH = OUT_F // 512             # 2 output halves
WS = 64.0                     # fp8 weight pre-scale

F32 = mybir.dt.float32
BF16 = mybir.dt.bfloat16
FP8 = mybir.dt.float8e4
AF = mybir.ActivationFunctionType
OP = mybir.AluOpType
PM = mybir.MatmulPerfMode

# balance knobs: per-icpair-index (0..3)
SQ_ACT = (5, 2, 2, 2)    # leading g's whose u^2 is computed by ACT Square
NB_POOL = (7, 6, 5, 4)   # leading g's whose v'*e basis op runs on GPSIMD
EPI_PRIO = 80            # schedule epilogue ops as if emitted ~2 chains earlier
USE_LN = False           # v' via LN_BWD_DX custom op (1 DVE op/slice + shared q)

_BUILD_CACHE = {}


def _build_nc(a_g, b_g, thr, osc, has_bias):
    nc = bacc.Bacc("TRN2", target_bir_lowering=False, debug=False,
                   num_devices=N_CORES)

    xT = nc.dram_tensor("xT", [IN_F, TPC], F32, kind="ExternalInput")
    ww = nc.dram_tensor("ww", [NPI * GRID, 128, 2, OUT_F], FP8,
                        kind="ExternalInput")
    wsn = nc.dram_tensor("wsn", [NIC, 128, OUT_F], BF16, kind="ExternalInput")
    negi = nc.dram_tensor("negi", [128, 128], BF16, kind="ExternalInput")
    bias = nc.dram_tensor("bias", [1, OUT_F], BF16, kind="ExternalInput")
    out = nc.dram_tensor("out", [TPC, OUT_F], BF16, kind="ExternalOutput")

    t64 = abs(WS * osc) * thr       # clamp bound in psum units
    inv = 1.0 / WS                  # final psum -> out scale

    with tile.TileContext(nc) as tc:
        with (
            tc.tile_pool(name="wpool", bufs=1) as wpool,
            tc.tile_pool(name="xpool", bufs=3) as xpool,
            tc.tile_pool(name="thpool", bufs=4) as thpool,
            tc.tile_pool(name="slpool", bufs=7) as slpool,
            tc.tile_pool(name="upool", bufs=5) as upool,
            tc.tile_pool(name="vpool", bufs=2) as vpool,
            tc.tile_pool(name="epool", bufs=2) as epool,
            tc.tile_pool(name="nbpool", bufs=4) as nbpool,
            tc.tile_pool(name="tpool", bufs=3) as tpool,
            tc.tile_pool(name="opool", bufs=4) as opool,
            tc.tile_pool(name="psum", bufs=8, space="PSUM") as pp,
        ):
            _consts = {}

            def const_col(val):
                val = float(val)
                if val not in _consts:
                    t = wpool.tile([128, 1], F32, name=f"const{len(_consts)}")
                    nc.gpsimd.memset(t[:], val)
                    _consts[val] = t
                return _consts[val][:]

            # ---- resident weights (SP queue; emitted after first x DMAs) ----
            ww_sb = wpool.tile([128, NPI * GRID, 2, OUT_F], FP8, name="ww_sb")
            wsn_sb = wpool.tile([128, NIC, OUT_F], BF16, name="wsn_sb")
            negi_sb = wpool.tile([128, 128], BF16, name="negi_sb")
            if has_bias:
                bias_sb = wpool.tile([1, OUT_F], BF16, name="bias_sb")
                ones_sb = wpool.tile([1, 128], BF16, name="ones_sb")
                nc.vector.memset(ones_sb[:], 1.0)

            def emit_weight_dmas(stage):
                # staged so early x prefetches aren't stuck behind ~20us of
                # weight traffic on the FIFO hwdge queue
                p0 = stage * GRID
                for p in range(p0, min(p0 + GRID, NPI * GRID)):
                    nc.sync.dma_start(out=ww_sb[:, p], in_=ww.ap()[p])
                if stage == NPI - 1:
                    nc.sync.dma_start(out=negi_sb[:], in_=negi.ap())
                    for j in range(NIC):
                        nc.sync.dma_start(out=wsn_sb[:, j], in_=wsn.ap()[j])
                    if has_bias:
                        nc.sync.dma_start(out=bias_sb[:], in_=bias.ap())

            x_t, sl_t, v_t, e_t, nb_t, psk_t = {}, {}, {}, {}, {}, {}
            GSPLIT = 4   # basis g-chunks: part 0 = g<4, part 1 = g>=4

            def emit_x(g_idx):
                """DMA the two x chunks of chain g_idx (ACT hwdge queue)."""
                st, i = g_idx // NPI, g_idx % NPI
                s0 = st * ST
                xp = xpool.tile([128, 2, ST], F32, tag="x", name=f"x{g_idx}")
                nc.scalar.dma_start(
                    out=xp[:],
                    in_=xT.ap()[2 * i * 128:(2 * i + 2) * 128, s0:s0 + ST]
                    .rearrange("(c p) t -> p c t", c=2))
                x_t[g_idx] = xp

            def emit_prep(g_idx):
                """tanh + 2*silu prep for chain g_idx (one ACT table only)."""
                if g_idx + 2 < NCH:
                    emit_x(g_idx + 2)
                xp = x_t.pop(g_idx)
                th = thpool.tile([128, 2, ST], BF16, tag="th", name=f"th{g_idx}")
                nc.scalar.activation(th[:], xp[:], AF.Tanh, scale=1.0)
                th2 = upool.tile([128, 2, ST], BF16, tag="u", name=f"th2_{g_idx}")
                nc.scalar.activation(th2[:], xp[:], AF.Tanh, scale=0.5)
                sl = slpool.tile([128, 2, ST], BF16, tag="sl", name=f"sl{g_idx}")
                # 2*silu(x) = (tanh(x/2)+1)*x ; wsn carries the 1/2
                nc.vector.scalar_tensor_tensor(sl[:], th2[:], 1.0, xp[:],
                                               OP.add, OP.mult)
                sl_t[g_idx] = sl
                v = vpool.tile([128, GRID, 2, ST], BF16, tag="v", name=f"v{g_idx}")
                e = epool.tile([128, GRID, 2, ST], BF16, tag="e", name=f"e{g_idx}")
                nb = nbpool.tile([128, GRID, 2, ST], FP8, tag="nb",
                                 name=f"nb{g_idx}")
                v_t[g_idx], e_t[g_idx], nb_t[g_idx] = v, e, nb
                return th

            def emit_vprime(g_idx, th):
                """v' = u^2-1 for all 7 wavelets of chain g_idx."""
                i = g_idx % NPI
                v = v_t[g_idx]
                q = None
                for g in range(GRID):
                    if g < SQ_ACT[i]:
                        vt = upool.tile([128, 2, ST], BF16, tag="u",
                                        name=f"vt{g_idx}_{g}")
                        nc.scalar.activation(vt[:], th[:], AF.Square,
                                             scale=a_g[g], bias=const_col(b_g[g]))
                        nc.vector.tensor_scalar(v[:, g], vt[:], 1.0, None,
                                                OP.subtract)
                    elif USE_LN:
                        # v' = (q - C0*th - C1)*a^2 with q = th^2 shared
                        if q is None:
                            q = upool.tile([128, 2, ST], BF16, tag="u",
                                           name=f"q{g_idx}")
                            nc.vector.tensor_tensor(q[:], th[:], th[:], OP.mult)
                        a, b = a_g[g], b_g[g]
                        nc.vector.ln_bwd_dx(
                            v[:, g].rearrange("p a b -> p (a b)"),
                            q[:].rearrange("p a b -> p (a b)"),
                            th[:].rearrange("p a b -> p (a b)"),
                            -2.0 * b / a, (1.0 - b * b) / (a * a),
                            scale=a * a)
                    else:
                        # u^2-1 = (u+1)(u-1), both factors affine in th
                        up = upool.tile([128, 2, ST], BF16, tag="u",
                                        name=f"up{g_idx}_{g}")
                        um = upool.tile([128, 2, ST], BF16, tag="u",
                                        name=f"um{g_idx}_{g}")
                        nc.vector.tensor_scalar(up[:], th[:], a_g[g],
                                                b_g[g] + 1.0, OP.mult, OP.add)
                        nc.vector.tensor_scalar(um[:], th[:], a_g[g],
                                                b_g[g] - 1.0, OP.mult, OP.add)
                        nc.vector.tensor_tensor(v[:, g], up[:], um[:], OP.mult)

            def emit_basis(g_idx, part):
                """e = exp(-u^2/2), nb = v'*e for one g-chunk (2-g pieces so
                downstream matmuls can start early)."""
                i = g_idx % NPI
                g0, g1 = (0, GSPLIT) if part == 0 else (GSPLIT, GRID)
                v, e, nb = v_t[g_idx], e_t[g_idx], nb_t[g_idx]
                k = min(max(NB_POOL[i], g0), g1)   # pool takes g in [g0, k)
                chunks = []
                c0 = g0
                while c0 < g1:
                    c1 = g1 if c0 + 3 >= g1 else c0 + 2
                    chunks.append((c0, c1))
                    c0 = c1
                for c0, c1 in chunks:
                    # e^{-1/2} of exp(-(u^2-1)/2) is folded into ww host-side
                    nc.scalar.activation(e[:, c0:c1], v[:, c0:c1], AF.Exp,
                                         scale=-0.5)
                    p1 = min(c1, k)
                    if p1 > c0:
                        nc.gpsimd.tensor_tensor(nb[:, c0:p1], v[:, c0:p1],
                                                e[:, c0:p1], OP.mult)
                    d0 = max(c0, k)
                    if d0 < c1:
                        nc.vector.tensor_tensor(nb[:, d0:c1], v[:, d0:c1],
                                                e[:, d0:c1], OP.mult)

            def emit_wavelet(g_idx, part):
                """DoubleRow matmuls of one g-chunk into the 8 psum banks."""
                st, i = g_idx // NPI, g_idx % NPI
                gs = range(0, GSPLIT) if part == 0 else range(GSPLIT, GRID)
                nb = nb_t[g_idx]
                if i == 0 and part == 0:
                    for tt in range(NTT):
                        for h in range(NH):
                            psk_t[(st, tt, h)] = pp.tile(
                                [128, 512], F32, tag="psk",
                                name=f"psk{st}_{tt}_{h}")
                for g in gs:
                    for tt in range(NTT):
                        for h in range(NH):
                            nc.tensor.matmul(
                                psk_t[(st, tt, h)][:],
                                nb[:, g, :, tt * 128:(tt + 1) * 128],
                                ww_sb[:, i * GRID + g, :, h * 512:h * 512 + 512],
                                start=(i == 0 and g == 0), stop=False,
                                perf_mode=PM.DoubleRow)
                if part == 1:
                    nb_t.pop(g_idx)

            def emit_wavelet_tt(g_idx, tt):
                """Last chain of a super-tile: all 7 g's for one token tile."""
                st, i = g_idx // NPI, g_idx % NPI
                nb = nb_t[g_idx]
                for g in range(GRID):
                    for h in range(NH):
                        nc.tensor.matmul(
                            psk_t[(st, tt, h)][:],
                            nb[:, g, :, tt * 128:(tt + 1) * 128],
                            ww_sb[:, i * GRID + g, :, h * 512:h * 512 + 512],
                            start=False, stop=False,
                            perf_mode=PM.DoubleRow)

            def emit_epi_tt(st, tt):
                """clamp -> -I fold -> base matmuls -> scale-out for one tt."""
                s0 = st * ST
                t_ = {}
                for h in range(NH):
                    psk = psk_t[(st, tt, h)]
                    t_[h] = tpool.tile([128, 512], BF16, tag="t",
                                       name=f"t{st}_{tt}_{h}")
                    nc.vector.tensor_scalar(t_[h][:], psk[:], -t64, t64,
                                            OP.max, OP.min)
                for h in range(NH):
                    psk = psk_t[(st, tt, h)]
                    nc.tensor.matmul(psk[:], negi_sb[:], t_[h][:],
                                     start=False, stop=False)
                    for ic in range(NIC):
                        sl = sl_t[st * NPI + ic // 2]
                        last = (ic == NIC - 1 and not has_bias)
                        nc.tensor.matmul(
                            psk[:], sl[:, ic % 2, tt * 128:(tt + 1) * 128],
                            wsn_sb[:, ic, h * 512:h * 512 + 512],
                            start=False, stop=last)
                    if has_bias:
                        nc.tensor.matmul(psk[:], ones_sb[:],
                                         bias_sb[:, h * 512:h * 512 + 512],
                                         start=False, stop=True)
                for h in range(NH):
                    psk = psk_t.pop((st, tt, h))
                    og = opool.tile([128, 512], BF16, tag="og",
                                    name=f"o{st}_{tt}_{h}")
                    nc.vector.tensor_scalar(og[:], psk[:], inv, None, OP.mult)
                    nc.sync.dma_start(
                        out=out.ap()[s0 + tt * 128:s0 + (tt + 1) * 128,
                                     h * 512:h * 512 + 512],
                        in_=og[:])

            # ---- software-pipelined schedule (lookahead 2 chains) ----
            emit_x(0)
            emit_x(1)
            for g_idx in range(NCH + 3):
                if g_idx < NPI:
                    emit_weight_dmas(g_idx)
                if g_idx < NCH:
                    th = emit_prep(g_idx)
                    emit_vprime(g_idx, th)
                    emit_basis(g_idx, 0)
                w = g_idx - 3
                if w >= 0:
                    st, i = w // NPI, w % NPI
                    emit_wavelet(w, 0)
                    emit_wavelet(w, 1)
                    if i == NPI - 1:
                        with tc.high_priority(offset=EPI_PRIO):
                            for tt in range(NTT):
                                emit_epi_tt(st, tt)
                        for j in range(NPI):
                            sl_t.pop(st * NPI + j)
                if g_idx < NCH:
                    emit_basis(g_idx, 1)
    nc.compile()
    return nc


def kernel(x, base_w, base_b, u, translation, scale, wavelet_w, soft_threshold,
           output_scale):
    x = np.asarray(x, np.float32)
    base_w = np.asarray(base_w, np.float32)
    base_b = np.asarray(base_b, np.float32)
    u = np.asarray(u, np.float32)
    translation = np.asarray(translation, np.float32).reshape(-1)
    scale = np.asarray(scale, np.float32).reshape(-1)
    wavelet_w = np.asarray(wavelet_w, np.float32)
    thr = float(np.log1p(np.exp(np.float32(soft_threshold.reshape(-1)[0]))))
    osc = float(np.asarray(output_scale).reshape(-1)[0])

    # spectral norm (one power iteration, no-grad buffers) on host: O(IN*OUT)
    def l2n(v):
        return v / (np.linalg.norm(v) + np.float32(1e-12))
    v = l2n(base_w.T @ u)
    u2 = l2n(base_w @ v)
    sigma = u2 @ (base_w @ v)
    w_sn = base_w / sigma

    safe_s = np.maximum(np.abs(scale), np.float32(0.1))
    a_g = tuple(float(2.5 / safe_s[g]) for g in range(GRID))
    b_g = tuple(float(-translation[g] / safe_s[g]) for g in range(GRID))
    has_bias = bool(np.any(base_b != 0))

    key = (a_g, b_g, thr, osc, has_bias)
    if key not in _BUILD_CACHE:
        _BUILD_CACHE[key] = _build_nc(a_g, b_g, thr, osc, has_bias)
    nc = _BUILD_CACHE[key]

    bf16 = ml_dtypes.bfloat16
    f8 = ml_dtypes.float8_e4m3
    # wsn[ic, f, o] = 0.5*64*osc*w_sn[o, ic*128+f]  (0.5 pairs with 2*silu)
    wsn_h = np.ascontiguousarray(
        (0.5 * WS * osc * w_sn.T).reshape(NIC, 128, OUT_F).astype(bf16))
    # ww[(i,g), f, c, o] = -64*osc*wavelet_w[o, ((2i+c)*128+f)*GRID+g]
    w3 = wavelet_w.reshape(OUT_F, IN_F, GRID)            # [o, fi, g]
    wt = w3.transpose(1, 2, 0)                           # [fi, g, o]
    wt = wt.reshape(NPI, 2, 128, GRID, OUT_F)            # [i, c, f, g, o]
    # exp(-1/2) folds the basis e^{-(u^2-1)/2} back to e^{-u^2/2}
    ww_h = np.ascontiguousarray(
        (-WS * osc * float(np.exp(-0.5))) * wt.transpose(0, 3, 2, 1, 4)
        .reshape(NPI * GRID, 128, 2, OUT_F)).astype(f8)
    negi_h = np.ascontiguousarray((-np.eye(128, dtype=np.float32)).astype(bf16))
    bias_h = np.ascontiguousarray(
        (WS * osc * base_b).reshape(1, OUT_F).astype(bf16))

    x_flat = x.reshape(NTOK, IN_F)
    in_maps = []
    for c in range(N_CORES):
        xc = x_flat[c * TPC:(c + 1) * TPC]               # [TPC, IN_F]
        in_maps.append({
            "xT": np.ascontiguousarray(xc.T),            # [IN_F, TPC] f32
            "ww": ww_h,
            "wsn": wsn_h,
            "negi": negi_h,
            "bias": bias_h,
        })

    res = run_bass_kernel_spmd(nc, in_maps, core_ids=list(range(N_CORES)))
    out = np.concatenate([res.results[c]["out"].astype(np.float32)
                          for c in range(N_CORES)], axis=0)
    return out.reshape(BATCH, SEQ, OUT_F)
